# revision 52
# baseline (speedup 1.0000x reference)
"""Causal multi-head attention (B=4, H=16, S=2048, D=64) on 8 TRN2 NeuronCores.

Sharding: B*H = 64 heads, 8 heads per core (data/head parallel, no comms).

Per-core pipeline (per head):
  - DMA Q,K,V [2048,64] f32 -> SBUF, cast to bf16 (DVE)
  - transpose Q,K to d-major via single-tile PE transposes ([128,64] ->
    [64,128]: output always lands on partitions 0:64, so no partition-
    shift DMA), flatten to [128, 2048] with the top 64 partitions zeroed
    (K=128 contraction pad: K=64 matmuls under-occupy the PE array and
    trip HAM re-throttling to 1.2GHz -- measured 1.6x whole-kernel hit)
  - QK^T emitted as a uniform stream of 512-col chunks of the concatenated
    causal strips (strip j = E^T[k-tile j, q >= 128j], width 2048-128j;
    17408 stream cols/head).  Chunks land in a rotating 2x3-bank PSUM ring;
    chunks that span strip boundaries become 2-3 matmul pieces.
  - exp via ScalarE in ONE ACTIVATE per 3 chunks (FD=1536; 12 instr/head
    instead of 24) writing a contiguous per-head at_all bf16 buffer.
    ScalarE is the bottleneck engine (1 elem/lane/cycle @1.2GHz + ~180cy
    per-instruction overhead), so maximizing FD per ACTIVATE is the win.
  - causal zeroing of each strip's diagonal tile via DVE trimask multiply
  - A@V with a ones-column appended to V: O[q,0:64] = sum_k A^T_k.T @ V_k,
    O[q,64] = softmax denominator; emitted with a group lag + per-group
    matmul cap so trailing A@V trains spread over the next head's groups;
    normalize with VectorE reciprocal + scale
  - head 0 runs strips 8-15 first (needs only the second DMA split), the
    last head schedules A@V greedily (LAG=0, no cap) to shrink the drain
  - the short remainder exp group sits second-to-last so the ring buffer
    the next head's first chunks WAR on is released early, and those
    chunks are pre-emitted before the current head's last group -- the
    exp stream crosses head boundaries with only ~330ns of sem latency

PSUM budget: exp ring 2x3 banks + transpose staging 1 + A@V accum 1 = 8.
(Tighter packing is blocked by bank-granular PSUM allocation and by
start=True matmuls clearing has_written for their whole bank, which
forbids co-locating anything with the accumulating A@V bank.)
"""

import os
import sys

try:
    import concourse.bass as bass  # noqa: F401
except ImportError:
    sys.path.insert(0, "/opt/trn_rl_repo")

import numpy as np

import concourse.mybir as mybir
import concourse.tile as tile
from concourse import bacc
from concourse.bass_utils import run_bass_kernel_spmd
from concourse.masks import make_identity

B, H, S, D = 4, 16, 2048, 64
N_CORES = 8
HEADS = B * H
HPC = HEADS // N_CORES  # heads per core
P = 128
ST = S // P  # 16 s-tiles per head

F32 = mybir.dt.float32
BF16 = mybir.dt.bfloat16

SCALE = 1.0 / float(np.sqrt(D))

CH = 512                       # stream chunk width (one PSUM bank)
GRP = int(os.environ.get("K_GRP", "3"))   # chunks per exp ACTIVATE
LAG = int(os.environ.get("K_LAG", "1"))   # A@V lag in groups
CAP = int(os.environ.get("K_CAP", "10"))  # max A@V matmuls scheduled per group
PREP_G = int(os.environ.get("K_PREPG", "4"))  # group idx to start next head's prep
# (4 keeps the prep's DVE/DMA burst clear of the late groups where the
# trailing A@V trains cluster -- measured fewer exp-stream gaps than 6)

# strip geometry: strip j covers q in [128j, 2048), stream-concatenated.
# Head 0 processes strips 8-15 first (exactly 9 chunks = 3 exp groups that
# need only the second halves of Q/K), so its first exp starts after one
# DMA split instead of the whole head load.
W_STRIP = [S - P * j for j in range(ST)]
STREAM = sum(W_STRIP)                         # 17408
NCH = STREAM // CH                            # 34 chunks per head
# exp groups per head: GRP-chunk groups plus a short remainder group.
# Mid heads put the short group second-to-last: the next head's first
# chunks WAR on the ring buffer released by group NGRP-2's exp, so a
# short exp there frees it early and the exp stream crosses the head
# boundary without a gap.  The last head has no successor and keeps the
# short group last, so its final exp is short and the trailing A@V
# trains (gated by the final group's masks) start ~1us earlier.
GROUPS = {"last": []}
_c = 0
while _c < NCH:
    GROUPS["last"].append((_c, min(_c + GRP, NCH)))
    _c += GRP
GROUPS["mid"] = list(GROUPS["last"])
if len(GROUPS["mid"]) >= 2 and NCH % GRP:
    _a, _b = GROUPS["mid"][-2]
    _rem = NCH % GRP
    GROUPS["mid"][-2] = (_a, _a + _rem)
    GROUPS["mid"][-1] = (_a + _rem, NCH)
NGRP = len(GROUPS["mid"])                     # 12 for GRP=3
GROUP_OF_CHUNK = {}
for _gk, _ranges in GROUPS.items():
    m = {}
    for _g, (_a, _b) in enumerate(_ranges):
        for _cc in range(_a, _b):
            m[_cc] = _g
    GROUP_OF_CHUNK[_gk] = m

ORDERS = {
    "h0": list(range(8, ST)) + list(range(0, 8)),
    "n": list(range(ST)),
}
OFFK = {}  # kind -> strip id -> stream offset
for _kind, _order in ORDERS.items():
    off = {}
    pos = 0
    for _j in _order:
        off[_j] = pos
        pos += W_STRIP[_j]
    OFFK[_kind] = off


def head_kind(h):
    return "h0" if h == 0 else "n"


def chunk_pieces(kind, c):
    """Matmul pieces of stream chunk c: list of (strip j, q0, ring_col, w)."""
    lo, hi = CH * c, CH * (c + 1)
    out = []
    for j in ORDERS[kind]:
        a = max(lo, OFFK[kind][j])
        b = min(hi, OFFK[kind][j] + W_STRIP[j])
        if a < b:
            out.append((j, P * j + (a - OFFK[kind][j]), a - lo, b - a))
    return out


def ready_group(kind, gk, jq):
    """Exp group after which A@V for q-tile jq can run: all needed at_all
    regions (strip k's tile jq-k, for k<=jq) exp'd and the diagonal masked."""
    pos = max(OFFK[kind][k] + P * (jq - k) + P for k in range(jq + 1))
    return GROUP_OF_CHUNK[gk][(pos - 1) // CH]


def build_nc(heads_per_core=HPC):
    nc = bacc.Bacc("TRN2", target_bir_lowering=False, debug=False,
                   num_devices=N_CORES)
    q_d = nc.dram_tensor("Q", [heads_per_core, S, D], F32, kind="ExternalInput")
    k_d = nc.dram_tensor("K", [heads_per_core, S, D], F32, kind="ExternalInput")
    v_d = nc.dram_tensor("V", [heads_per_core, S, D], F32, kind="ExternalInput")
    o_d = nc.dram_tensor("out", [heads_per_core, S, D], F32, kind="ExternalOutput")

    with tile.TileContext(nc) as tc:
        with (
            tc.tile_pool(name="const", bufs=1) as const,
            tc.tile_pool(name="stage", bufs=2) as stage,
            tc.tile_pool(name="bfp", bufs=2) as bfp,
            tc.tile_pool(name="tp", bufs=2) as tpool,
            tc.tile_pool(name="atp", bufs=2) as atp,
            tc.tile_pool(name="osb", bufs=2) as osbp,
            tc.tile_pool(name="small", bufs=8) as small,
            tc.tile_pool(name="ps", bufs=1, space="PSUM") as ps,
        ):
            identity = const.tile([P, P], BF16, tag="ident")
            make_identity(nc, identity)
            # upper-triangular (incl. diagonal) ones: keep q >= k
            trimask = const.tile([P, P], BF16, tag="trimask")
            nc.gpsimd.memset(trimask, 1.0)
            nc.gpsimd.affine_select(
                out=trimask, in_=trimask,
                compare_op=mybir.AluOpType.is_ge,
                fill=0.0, base=0,
                pattern=[[1, P]], channel_multiplier=-1,
            )

            def emit_prep(h, nsplit=1):
                """Load + cast + transpose head h's operands. Returns the
                tiles the chunk loop needs (qT, kT flat d-major; v_aug).
                qT3/kT3 top halves are zero (K=128 contraction pad: K=64
                matmuls under-occupy the PE and trip HAM re-throttling)."""
                q_raw = stage.tile([P, ST, D], F32, tag="qraw")
                k_raw = stage.tile([P, ST, D], F32, tag="kraw")
                v_raw = stage.tile([P, ST, D], F32, tag="vraw")
                qT3 = tpool.tile([P, ST, P], BF16, tag="qT3")
                kT3 = tpool.tile([P, ST, P], BF16, tag="kT3")
                q_bf = bfp.tile([P, ST, D], BF16, tag="qbf")
                k_bf = bfp.tile([P, ST, D], BF16, tag="kbf")
                if h < 2:  # pool slots keep their zero top halves across heads
                    nc.gpsimd.memset(qT3[64:P, :, :], 0.0)
                    nc.gpsimd.memset(kT3[64:P, :, :], 0.0)
                splits = [(ST * i // nsplit, ST * (i + 1) // nsplit)
                          for i in range(nsplit)]
                if h == 0:
                    # strips 8-15 run first.  Their tiles load in two 4-tile
                    # sub-chains (the only deps of exp groups 0-2, so the
                    # first exp starts sooner); the 0-7 half stays one chain
                    # issued right behind, keeping group 3+ deps on time.
                    splits = [(8, 12), (12, 16), (0, 8)]
                for s0, s1 in splits:
                    for (raw, d_) in ((q_raw, q_d), (k_raw, k_d)):
                        nc.sync.dma_start(
                            out=raw[:, s0:s1, :],
                            in_=d_[h].rearrange("(b p) d -> p b d", p=P)[:, s0:s1, :])
                for si, (s0, s1) in enumerate(splits):
                    if si == min(1, nsplit - 1):
                        # defer V out of the first in-flight DMA window
                        nc.sync.dma_start(
                            out=v_raw, in_=v_d[h].rearrange("(b p) d -> p b d", p=P))
                    chain = (
                        (q_raw, q_bf, qT3),
                        (k_raw, k_bf, kT3),
                    )
                    # phase 1: casts (DVE)
                    for (raw, bf_, t3) in chain:
                        nc.vector.tensor_copy(bf_[:, s0:s1, :], raw[:, s0:s1, :])
                    # phase 2+3: single-tile PE transposes [128,64] -> [64,128]
                    # (output always lands at partitions 0:64 -- no partition-
                    # shift DMA needed for odd tiles), then one flatten copy
                    # per round of <=8 tiles
                    for (raw, bf_, t3) in chain:
                        for r0 in range(s0, s1, 8):
                            r1 = min(r0 + 8, s1)
                            tp_ps = ps.tile([64, 8, P], BF16, tag="tp",
                                            bufs=1, name="tp_ps")
                            for t in range(r0, r1):
                                nc.tensor.transpose(
                                    tp_ps[:, t - r0, :], bf_[:, t, :], identity)
                            nc.vector.tensor_copy(t3[0:64, r0:r1, :],
                                                  tp_ps[:, 0:r1 - r0, :])
                v_aug = bfp.tile([P, ST, D + 1], BF16, tag="vaug")
                nc.vector.tensor_copy(v_aug[:, :, 0:D], v_raw)
                nc.vector.memset(v_aug[:, :, D:D + 1], 1.0)
                return (qT3.rearrange("p t c -> p (t c)"),
                        kT3.rearrange("p t c -> p (t c)"), v_aug)

            # Per-head pipeline state; two heads live at once.
            state = {}

            def alloc_ring(h, g):
                return ps.tile([P, GRP, CH], F32, tag="ring", bufs=2,
                               name=f"ring_{h}_{g}")

            def group_kind(h):
                return "last" if h == heads_per_core - 1 else "mid"

            def fill_ring(h, g, ring):
                """QK^T chunk matmuls of group g into the PSUM ring."""
                st = state[h]
                kind = head_kind(h)
                qT, kT = st["qT"], st["kT"]
                c0, c1 = GROUPS[group_kind(h)][g]
                for c in range(c0, c1):
                    for (j, qg, rcol, w) in chunk_pieces(kind, c):
                        nc.tensor.matmul(
                            ring[:, c - c0, rcol:rcol + w],
                            lhsT=kT[:, P * j:P * (j + 1)],
                            rhs=qT[:, qg:qg + w],
                            start=True, stop=True,
                        )

            def emit_exp(h, g, ring):
                """One exp over group g's ring + causal masks it unlocks."""
                st = state[h]
                kind = head_kind(h)
                at_all = st["at"]
                c0, c1 = GROUPS[group_kind(h)][g]
                nch = c1 - c0
                nc.scalar.activation(
                    at_all[:, CH * c0:CH * c1],
                    ring[:, 0:nch, :].rearrange("p a b -> p (a b)"),
                    mybir.ActivationFunctionType.Exp,
                    scale=SCALE,
                )
                # causal mask for strips whose diagonal tile this group covers
                for j in range(ST):
                    if c0 <= OFFK[kind][j] // CH < c1:
                        nc.vector.tensor_mul(
                            at_all[:, OFFK[kind][j]:OFFK[kind][j] + P],
                            at_all[:, OFFK[kind][j]:OFFK[kind][j] + P],
                            trimask,
                        )

            def emit_group(h, g):
                ring = alloc_ring(h, g)
                fill_ring(h, g, ring)
                emit_exp(h, g, ring)

            def emit_av(h, jq):
                """A@V for q-tile jq of head h; groups of four q-tiles share
                one PSUM bank + one batched normalize; stream output DMA."""
                st = state[h]
                at_all, v_aug, o_sb = st["at"], st["v_aug"], st["o_sb"]
                if jq % 4 == 0:
                    st["o4"] = ps.tile([P, 4, D + 1], F32, tag="o",
                                       bufs=1, name="o4")
                o4 = st["o4"]
                offk = OFFK[head_kind(h)]
                for k in range(jq + 1):
                    a0 = offk[k] + P * (jq - k)
                    nc.tensor.matmul(
                        o4[:, jq % 4, :],
                        lhsT=at_all[:, a0:a0 + P],
                        rhs=v_aug[:, k, :],
                        start=(k == 0), stop=(k == jq),
                    )
                if jq % 4 == 3:
                    recip4 = small.tile([P, 4], F32, tag="recip")
                    nc.vector.reciprocal(
                        recip4,
                        o4[:, :, D:D + 1].rearrange("p a b -> p (a b)"),
                    )
                    rb = bass.AP(tensor=recip4.tensor, offset=recip4.offset,
                                 ap=[recip4.ap[0], recip4.ap[1], [0, D]])
                    nc.vector.tensor_tensor(
                        out=o_sb[:, jq - 3:jq + 1, :],
                        in0=o4[:, :, 0:D], in1=rb,
                        op=mybir.AluOpType.mult,
                    )
                    nc.sync.dma_start(
                        out=o_d[h].rearrange("(b p) d -> p b d", p=P)
                                  [:, jq - 3:jq + 1, :],
                        in_=o_sb[:, jq - 3:jq + 1, :],
                    )
                if jq == ST - 1:
                    del state[h]

            # A@V slot schedule: q-tile jq is ready once the group holding
            # its diagonal chunk has exp'd+masked; add LAG groups, then
            # greedily cap matmuls per group so the PE load stays even.
            def av_slots(heads_n):
                slots = {}
                load = {}
                prev = 0
                for h in range(heads_n):
                    last = h == heads_n - 1
                    for jq in range(ST):
                        lag = 0 if last else LAG
                        gk = "last" if last else "mid"
                        ready = h * NGRP + ready_group(head_kind(h), gk, jq) + lag
                        s = max(ready, prev)
                        while (not last and load.get(s, 0)
                               and load.get(s, 0) + (jq + 1) > CAP):
                            s += 1
                        load[s] = load.get(s, 0) + (jq + 1)
                        slots[(h, jq)] = s
                        prev = s
                return slots

            slot = av_slots(heads_per_core)
            tasks = [(h, jq) for h in range(heads_per_core) for jq in range(ST)]

            qT0, kT0, v_aug0 = emit_prep(0, nsplit=int(os.environ.get("K_NSPLIT0", "2")))
            state[0] = {"qT": qT0, "kT": kT0, "v_aug": v_aug0,
                        "at": atp.tile([P, STREAM], BF16, tag="at_all",
                                       name="at0"),
                        "o_sb": osbp.tile([P, ST, D], F32, tag="osb", name="osb0")}
            av_next = 0
            prefetched = {}
            for G in range(heads_per_core * NGRP):
                h, g = divmod(G, NGRP)
                if (h, g) in prefetched:
                    # chunks were emitted before the previous head's last
                    # group; only the exp remains (keeps the ACT queue tight
                    # across the head boundary)
                    emit_exp(h, g, prefetched.pop((h, g)))
                elif g == NGRP - 1 and h + 1 < heads_per_core:
                    ring_last = alloc_ring(h, g)
                    ring_next = alloc_ring(h + 1, 0)
                    fill_ring(h, g, ring_last)
                    fill_ring(h + 1, 0, ring_next)
                    emit_exp(h, g, ring_last)
                    prefetched[(h + 1, 0)] = ring_next
                else:
                    emit_group(h, g)
                if g == PREP_G and h + 1 < heads_per_core:
                    qTn, kTn, v_augn = emit_prep(h + 1)
                    state[h + 1] = {
                        "qT": qTn, "kT": kTn, "v_aug": v_augn,
                        "at": atp.tile([P, STREAM], BF16, tag="at_all",
                                       name=f"at{h + 1}"),
                        "o_sb": osbp.tile([P, ST, D], F32, tag="osb",
                                          name=f"osb{h + 1}"),
                    }
                while av_next < len(tasks) and slot[tasks[av_next]] <= G:
                    emit_av(*tasks[av_next])
                    av_next += 1
            while av_next < len(tasks):
                emit_av(*tasks[av_next])
                av_next += 1

    nc.compile()
    return nc


_NC_CACHE = {}


def _get_nc(heads_per_core=HPC):
    if heads_per_core not in _NC_CACHE:
        _NC_CACHE[heads_per_core] = build_nc(heads_per_core)
    return _NC_CACHE[heads_per_core]


def run_sharded(Q, K, V, heads_per_core=HPC, **run_kwargs):
    """Q, K, V: [HEADS-or-subset, S, D] f32 flattened over (B, H)."""
    nc = _get_nc(heads_per_core)
    n = heads_per_core
    in_maps = [
        {
            "Q": np.ascontiguousarray(Q[i * n:(i + 1) * n]),
            "K": np.ascontiguousarray(K[i * n:(i + 1) * n]),
            "V": np.ascontiguousarray(V[i * n:(i + 1) * n]),
        }
        for i in range(N_CORES)
    ]
    last_err = None
    for attempt in range(3):
        try:
            res = run_bass_kernel_spmd(nc, in_maps,
                                       core_ids=list(range(N_CORES)),
                                       **run_kwargs)
            out = np.concatenate(
                [np.asarray(res.results[i]["out"]) for i in range(N_CORES)],
                axis=0)
            return out, res
        except Exception as e:  # transient NRT_EXEC_UNIT_UNRECOVERABLE etc.
            last_err = e
            import time
            time.sleep(2.0)
    raise last_err


def kernel(Q, K, V, mask=None):
    Q = np.asarray(Q, dtype=np.float32).reshape(HEADS, S, D)
    K = np.asarray(K, dtype=np.float32).reshape(HEADS, S, D)
    V = np.asarray(V, dtype=np.float32).reshape(HEADS, S, D)
    out, _ = run_sharded(Q, K, V)
    return out.reshape(B, H, S, D)


# revision 57
# speedup vs baseline: 1.1871x; 1.1871x over previous
"""Causal multi-head attention (B=4, H=16, S=2048, D=64) on 8 TRN2 NeuronCores.

Sharding: B*H = 64 heads, 8 heads per core (data/head parallel, no comms).

Per-core pipeline (per head):
  - DMA Q,K,V [2048,64] f32 -> SBUF, cast to bf16 (DVE)
  - transpose Q,K to d-major via single-tile PE transposes ([128,64] ->
    [64,128]: output always lands on partitions 0:64, so no partition-
    shift DMA), flatten to [128, 2048] with the top 64 partitions zeroed
    (K=128 contraction pad: K=64 matmuls under-occupy the PE array and
    trip HAM re-throttling to 1.2GHz -- measured 1.6x whole-kernel hit)
  - QK^T emitted as a uniform stream of 512-col chunks of the concatenated
    causal strips (strip j = E^T[k-tile j, q >= 128j], width 2048-128j;
    17408 stream cols/head).  Chunks land in a rotating 2x3-bank PSUM ring;
    chunks that span strip boundaries become 2-3 matmul pieces.
  - exp via ScalarE in ONE ACTIVATE per 3 chunks (FD=1536; 12 instr/head
    instead of 24) writing a contiguous per-head at_all bf16 buffer.
    ScalarE is the bottleneck engine (1 elem/lane/cycle @1.2GHz + ~180cy
    per-instruction overhead), so maximizing FD per ACTIVATE is the win.
  - causal zeroing of each strip's diagonal tile via DVE trimask multiply
  - A@V with a ones-column appended to V: O[q,0:64] = sum_k A^T_k.T @ V_k,
    O[q,64] = softmax denominator; emitted with a group lag + per-group
    matmul cap so trailing A@V trains spread over the next head's groups;
    normalize with VectorE reciprocal + scale
  - head 0 runs strips 8-15 first (needs only the second DMA split), the
    last head schedules A@V greedily (LAG=0, no cap) to shrink the drain
  - the short remainder exp group sits second-to-last so the ring buffer
    the next head's first chunks WAR on is released early, and those
    chunks are pre-emitted before the current head's last group -- the
    exp stream crosses head boundaries with only ~330ns of sem latency

PSUM budget: exp ring 2x3 banks + transpose staging 1 + A@V accum 1 = 8.
(Tighter packing is blocked by bank-granular PSUM allocation and by
start=True matmuls clearing has_written for their whole bank, which
forbids co-locating anything with the accumulating A@V bank.)
"""

import os
import sys

try:
    import concourse.bass as bass  # noqa: F401
except ImportError:
    sys.path.insert(0, "/opt/trn_rl_repo")

import numpy as np

import concourse.mybir as mybir
import concourse.tile as tile
from concourse import bacc
from concourse.bass_utils import run_bass_kernel_spmd
from concourse.masks import make_identity

B, H, S, D = 4, 16, 2048, 64
N_CORES = 8
HEADS = B * H
HPC = HEADS // N_CORES  # heads per core
P = 128
ST = S // P  # 16 s-tiles per head

F32 = mybir.dt.float32
BF16 = mybir.dt.bfloat16

SCALE = 1.0 / float(np.sqrt(D))

CH = 512                       # stream chunk width (one PSUM bank)
GRP = int(os.environ.get("K_GRP", "3"))   # chunks per exp ACTIVATE
LAG = int(os.environ.get("K_LAG", "1"))   # A@V lag in groups
CAP = int(os.environ.get("K_CAP", "10"))  # max A@V matmuls scheduled per group
PREP_G = int(os.environ.get("K_PREPG", "4"))  # group idx to start next head's prep
# (4 keeps the prep's DVE/DMA burst clear of the late groups where the
# trailing A@V trains cluster -- measured fewer exp-stream gaps than 6)

# strip geometry: strip j covers q in [128j, 2048), stream-concatenated.
# Head 0 processes strips 8-15 first (exactly 9 chunks = 3 exp groups that
# need only the second halves of Q/K), so its first exp starts after one
# DMA split instead of the whole head load.
W_STRIP = [S - P * j for j in range(ST)]
STREAM = sum(W_STRIP)                         # 17408
NCH = STREAM // CH                            # 34 chunks per head
# exp groups per head: GRP-chunk groups plus a short remainder group.
# Mid heads put the short group second-to-last: the next head's first
# chunks WAR on the ring buffer released by group NGRP-2's exp, so a
# short exp there frees it early and the exp stream crosses the head
# boundary without a gap.  The last head has no successor and keeps the
# short group last, so its final exp is short and the trailing A@V
# trains (gated by the final group's masks) start ~1us earlier.
GROUPS = {"last": []}
_c = 0
while _c < NCH:
    GROUPS["last"].append((_c, min(_c + GRP, NCH)))
    _c += GRP
GROUPS["mid"] = list(GROUPS["last"])
if len(GROUPS["mid"]) >= 2 and NCH % GRP:
    _a, _b = GROUPS["mid"][-2]
    _rem = NCH % GRP
    GROUPS["mid"][-2] = (_a, _a + _rem)
    GROUPS["mid"][-1] = (_a + _rem, NCH)
NGRP = len(GROUPS["mid"])                     # 12 for GRP=3
GROUP_OF_CHUNK = {}
for _gk, _ranges in GROUPS.items():
    m = {}
    for _g, (_a, _b) in enumerate(_ranges):
        for _cc in range(_a, _b):
            m[_cc] = _g
    GROUP_OF_CHUNK[_gk] = m

ORDERS = {
    "h0": list(range(8, ST)) + list(range(0, 8)),
    "n": list(range(ST)),
}
OFFK = {}  # kind -> strip id -> stream offset
for _kind, _order in ORDERS.items():
    off = {}
    pos = 0
    for _j in _order:
        off[_j] = pos
        pos += W_STRIP[_j]
    OFFK[_kind] = off


def head_kind(h):
    return "h0" if h == 0 else "n"


def chunk_pieces(kind, c):
    """Matmul pieces of stream chunk c: list of (strip j, q0, ring_col, w)."""
    lo, hi = CH * c, CH * (c + 1)
    out = []
    for j in ORDERS[kind]:
        a = max(lo, OFFK[kind][j])
        b = min(hi, OFFK[kind][j] + W_STRIP[j])
        if a < b:
            out.append((j, P * j + (a - OFFK[kind][j]), a - lo, b - a))
    return out


def ready_group(kind, gk, jq):
    """Exp group after which A@V for q-tile jq can run: all needed at_all
    regions (strip k's tile jq-k, for k<=jq) exp'd and the diagonal masked."""
    pos = max(OFFK[kind][k] + P * (jq - k) + P for k in range(jq + 1))
    return GROUP_OF_CHUNK[gk][(pos - 1) // CH]


def build_nc(heads_per_core=HPC):
    nc = bacc.Bacc("TRN2", target_bir_lowering=False, debug=False,
                   num_devices=N_CORES)
    q_d = nc.dram_tensor("Q", [heads_per_core, S, D], F32, kind="ExternalInput")
    k_d = nc.dram_tensor("K", [heads_per_core, S, D], F32, kind="ExternalInput")
    v_d = nc.dram_tensor("V", [heads_per_core, S, D], F32, kind="ExternalInput")
    o_d = nc.dram_tensor("out", [heads_per_core, S, D], F32, kind="ExternalOutput")

    with tile.TileContext(nc) as tc:
        with (
            tc.tile_pool(name="const", bufs=1) as const,
            tc.tile_pool(name="stage", bufs=2) as stage,
            tc.tile_pool(name="bfp", bufs=2) as bfp,
            tc.tile_pool(name="tp", bufs=2) as tpool,
            tc.tile_pool(name="atp", bufs=2) as atp,
            tc.tile_pool(name="osb", bufs=2) as osbp,
            tc.tile_pool(name="small", bufs=8) as small,
            tc.tile_pool(name="ps", bufs=1, space="PSUM") as ps,
        ):
            identity = const.tile([P, P], BF16, tag="ident")
            make_identity(nc, identity)
            # upper-triangular (incl. diagonal) ones: keep q >= k
            trimask = const.tile([P, P], BF16, tag="trimask")
            nc.gpsimd.memset(trimask, 1.0)
            nc.gpsimd.affine_select(
                out=trimask, in_=trimask,
                compare_op=mybir.AluOpType.is_ge,
                fill=0.0, base=0,
                pattern=[[1, P]], channel_multiplier=-1,
            )

            def emit_prep(h, nsplit=1):
                """Load + cast + transpose head h's operands. Returns the
                tiles the chunk loop needs (qT, kT flat d-major; v_aug).
                qT3/kT3 top halves are zero (K=128 contraction pad: K=64
                matmuls under-occupy the PE and trip HAM re-throttling)."""
                q_raw = stage.tile([P, ST, D], F32, tag="qraw")
                k_raw = stage.tile([P, ST, D], F32, tag="kraw")
                v_raw = stage.tile([P, ST, D], F32, tag="vraw")
                qT3 = tpool.tile([P, ST, P], BF16, tag="qT3")
                kT3 = tpool.tile([P, ST, P], BF16, tag="kT3")
                q_bf = bfp.tile([P, ST, D], BF16, tag="qbf")
                k_bf = bfp.tile([P, ST, D], BF16, tag="kbf")
                if h < 2:  # pool slots keep their zero top halves across heads
                    nc.gpsimd.memset(qT3[64:P, :, :], 0.0)
                    nc.gpsimd.memset(kT3[64:P, :, :], 0.0)
                splits = [(ST * i // nsplit, ST * (i + 1) // nsplit)
                          for i in range(nsplit)]
                if h == 0:
                    # strips 8-15 run first.  Their tiles load in two 4-tile
                    # sub-chains (the only deps of exp groups 0-2, so the
                    # first exp starts sooner); the 0-7 half stays one chain
                    # issued right behind, keeping group 3+ deps on time.
                    splits = [(8, 12), (12, 16), (0, 8)]
                for s0, s1 in splits:
                    for (raw, d_) in ((q_raw, q_d), (k_raw, k_d)):
                        nc.sync.dma_start(
                            out=raw[:, s0:s1, :],
                            in_=d_[h].rearrange("(b p) d -> p b d", p=P)[:, s0:s1, :])

                def emit_chain(s0, s1):
                    chain = (
                        (q_raw, q_bf, qT3),
                        (k_raw, k_bf, kT3),
                    )
                    # phase 1: casts (DVE)
                    for (raw, bf_, t3) in chain:
                        nc.vector.tensor_copy(bf_[:, s0:s1, :], raw[:, s0:s1, :])
                    # phase 2+3: single-tile PE transposes [128,64] -> [64,128]
                    # (output always lands at partitions 0:64 -- no partition-
                    # shift DMA needed for odd tiles), then one flatten copy
                    # per round of <=8 tiles
                    for (raw, bf_, t3) in chain:
                        for r0 in range(s0, s1, 8):
                            r1 = min(r0 + 8, s1)
                            tp_ps = ps.tile([64, 8, P], BF16, tag="tp",
                                            bufs=1, name="tp_ps")
                            for t in range(r0, r1):
                                nc.tensor.transpose(
                                    tp_ps[:, t - r0, :], bf_[:, t, :], identity)
                            nc.vector.tensor_copy(t3[0:64, r0:r1, :],
                                                  tp_ps[:, 0:r1 - r0, :])

                def emit_tail(lo_splits):
                    # the 0-7 half's cast/transpose chain + v_aug, emitted
                    # AFTER head 0's groups 0-2: its transposes wait on the
                    # lo DMA, and emitting them before the first chunk
                    # matmuls would head-of-line-block the PE queue
                    for (s0, s1) in lo_splits:
                        emit_chain(s0, s1)
                    nc.vector.tensor_copy(v_aug[:, :, 0:D], v_raw)
                    nc.vector.memset(v_aug[:, :, D:D + 1], 1.0)

                v_aug = bfp.tile([P, ST, D + 1], BF16, tag="vaug")
                nc.sync.dma_start(
                    out=v_raw, in_=v_d[h].rearrange("(b p) d -> p b d", p=P))
                if h == 0:
                    for (s0, s1) in splits[:2]:
                        emit_chain(s0, s1)
                    tail = lambda: emit_tail(splits[2:])  # noqa: E731
                else:
                    emit_tail(splits)
                    tail = None
                return (qT3.rearrange("p t c -> p (t c)"),
                        kT3.rearrange("p t c -> p (t c)"), v_aug, tail)

            # Per-head pipeline state; two heads live at once.
            state = {}

            def alloc_ring(h, g):
                return ps.tile([P, GRP, CH], F32, tag="ring", bufs=2,
                               name=f"ring_{h}_{g}")

            def group_kind(h):
                return "last" if h == heads_per_core - 1 else "mid"

            def fill_ring(h, g, ring):
                """QK^T chunk matmuls of group g into the PSUM ring."""
                st = state[h]
                kind = head_kind(h)
                qT, kT = st["qT"], st["kT"]
                c0, c1 = GROUPS[group_kind(h)][g]
                for c in range(c0, c1):
                    for (j, qg, rcol, w) in chunk_pieces(kind, c):
                        nc.tensor.matmul(
                            ring[:, c - c0, rcol:rcol + w],
                            lhsT=kT[:, P * j:P * (j + 1)],
                            rhs=qT[:, qg:qg + w],
                            start=True, stop=True,
                        )

            def emit_exp(h, g, ring):
                """One exp over group g's ring + causal masks it unlocks."""
                st = state[h]
                kind = head_kind(h)
                at_all = st["at"]
                c0, c1 = GROUPS[group_kind(h)][g]
                nch = c1 - c0
                nc.scalar.activation(
                    at_all[:, CH * c0:CH * c1],
                    ring[:, 0:nch, :].rearrange("p a b -> p (a b)"),
                    mybir.ActivationFunctionType.Exp,
                    scale=SCALE,
                )
                # causal mask for strips whose diagonal tile this group covers
                for j in range(ST):
                    if c0 <= OFFK[kind][j] // CH < c1:
                        nc.vector.tensor_mul(
                            at_all[:, OFFK[kind][j]:OFFK[kind][j] + P],
                            at_all[:, OFFK[kind][j]:OFFK[kind][j] + P],
                            trimask,
                        )

            def emit_group(h, g):
                ring = alloc_ring(h, g)
                fill_ring(h, g, ring)
                emit_exp(h, g, ring)

            def emit_av(h, jq):
                """A@V for q-tile jq of head h; groups of four q-tiles share
                one PSUM bank + one batched normalize; stream output DMA."""
                st = state[h]
                at_all, v_aug, o_sb = st["at"], st["v_aug"], st["o_sb"]
                if jq % 4 == 0:
                    st["o4"] = ps.tile([P, 4, D + 1], F32, tag="o",
                                       bufs=1, name="o4")
                o4 = st["o4"]
                offk = OFFK[head_kind(h)]
                for k in range(jq + 1):
                    a0 = offk[k] + P * (jq - k)
                    nc.tensor.matmul(
                        o4[:, jq % 4, :],
                        lhsT=at_all[:, a0:a0 + P],
                        rhs=v_aug[:, k, :],
                        start=(k == 0), stop=(k == jq),
                    )
                if jq % 4 == 3:
                    recip4 = small.tile([P, 4], F32, tag="recip")
                    nc.vector.reciprocal(
                        recip4,
                        o4[:, :, D:D + 1].rearrange("p a b -> p (a b)"),
                    )
                    rb = bass.AP(tensor=recip4.tensor, offset=recip4.offset,
                                 ap=[recip4.ap[0], recip4.ap[1], [0, D]])
                    nc.vector.tensor_tensor(
                        out=o_sb[:, jq - 3:jq + 1, :],
                        in0=o4[:, :, 0:D], in1=rb,
                        op=mybir.AluOpType.mult,
                    )
                    nc.sync.dma_start(
                        out=o_d[h].rearrange("(b p) d -> p b d", p=P)
                                  [:, jq - 3:jq + 1, :],
                        in_=o_sb[:, jq - 3:jq + 1, :],
                    )
                if jq == ST - 1:
                    del state[h]

            # A@V slot schedule: q-tile jq is ready once the group holding
            # its diagonal chunk has exp'd+masked; add LAG groups, then
            # greedily cap matmuls per group so the PE load stays even.
            def av_slots(heads_n):
                slots = {}
                load = {}
                prev = 0
                for h in range(heads_n):
                    last = h == heads_n - 1
                    for jq in range(ST):
                        lag = 0 if last else LAG
                        gk = "last" if last else "mid"
                        ready = h * NGRP + ready_group(head_kind(h), gk, jq) + lag
                        s = max(ready, prev)
                        while (not last and load.get(s, 0)
                               and load.get(s, 0) + (jq + 1) > CAP):
                            s += 1
                        load[s] = load.get(s, 0) + (jq + 1)
                        slots[(h, jq)] = s
                        prev = s
                return slots

            slot = av_slots(heads_per_core)
            tasks = [(h, jq) for h in range(heads_per_core) for jq in range(ST)]

            qT0, kT0, v_aug0, tail0 = emit_prep(0, nsplit=int(os.environ.get("K_NSPLIT0", "2")))
            state[0] = {"qT": qT0, "kT": kT0, "v_aug": v_aug0,
                        "at": atp.tile([P, STREAM], BF16, tag="at_all",
                                       name="at0"),
                        "o_sb": osbp.tile([P, ST, D], F32, tag="osb", name="osb0")}
            av_next = 0
            prefetched = {}
            for G in range(heads_per_core * NGRP):
                h, g = divmod(G, NGRP)
                if (h, g) in prefetched:
                    # chunks were emitted before the previous head's last
                    # group; only the exp remains (keeps the ACT queue tight
                    # across the head boundary)
                    emit_exp(h, g, prefetched.pop((h, g)))
                elif g == NGRP - 1 and h + 1 < heads_per_core:
                    ring_last = alloc_ring(h, g)
                    ring_next = alloc_ring(h + 1, 0)
                    fill_ring(h, g, ring_last)
                    fill_ring(h + 1, 0, ring_next)
                    emit_exp(h, g, ring_last)
                    prefetched[(h + 1, 0)] = ring_next
                else:
                    emit_group(h, g)
                if h == 0 and g == 2 and tail0 is not None:
                    tail0()  # head 0's lo-split chain, after groups 0-2
                if g == PREP_G and h + 1 < heads_per_core:
                    qTn, kTn, v_augn, _ = emit_prep(h + 1)
                    state[h + 1] = {
                        "qT": qTn, "kT": kTn, "v_aug": v_augn,
                        "at": atp.tile([P, STREAM], BF16, tag="at_all",
                                       name=f"at{h + 1}"),
                        "o_sb": osbp.tile([P, ST, D], F32, tag="osb",
                                          name=f"osb{h + 1}"),
                    }
                while av_next < len(tasks) and slot[tasks[av_next]] <= G:
                    emit_av(*tasks[av_next])
                    av_next += 1
            while av_next < len(tasks):
                emit_av(*tasks[av_next])
                av_next += 1

    nc.compile()
    return nc


_NC_CACHE = {}


def _get_nc(heads_per_core=HPC):
    if heads_per_core not in _NC_CACHE:
        _NC_CACHE[heads_per_core] = build_nc(heads_per_core)
    return _NC_CACHE[heads_per_core]


def run_sharded(Q, K, V, heads_per_core=HPC, **run_kwargs):
    """Q, K, V: [HEADS-or-subset, S, D] f32 flattened over (B, H)."""
    nc = _get_nc(heads_per_core)
    n = heads_per_core
    in_maps = [
        {
            "Q": np.ascontiguousarray(Q[i * n:(i + 1) * n]),
            "K": np.ascontiguousarray(K[i * n:(i + 1) * n]),
            "V": np.ascontiguousarray(V[i * n:(i + 1) * n]),
        }
        for i in range(N_CORES)
    ]
    last_err = None
    for attempt in range(3):
        try:
            res = run_bass_kernel_spmd(nc, in_maps,
                                       core_ids=list(range(N_CORES)),
                                       **run_kwargs)
            out = np.concatenate(
                [np.asarray(res.results[i]["out"]) for i in range(N_CORES)],
                axis=0)
            return out, res
        except Exception as e:  # transient NRT_EXEC_UNIT_UNRECOVERABLE etc.
            last_err = e
            import time
            time.sleep(2.0)
    raise last_err


def kernel(Q, K, V, mask=None):
    Q = np.asarray(Q, dtype=np.float32).reshape(HEADS, S, D)
    K = np.asarray(K, dtype=np.float32).reshape(HEADS, S, D)
    V = np.asarray(V, dtype=np.float32).reshape(HEADS, S, D)
    out, _ = run_sharded(Q, K, V)
    return out.reshape(B, H, S, D)
